# revision 34
# baseline (speedup 1.0000x reference)
"""Trainium2 Bass kernel for AdjStackAttentionWeights.

reference:  out = einsum('bsij,hs->bhij', stacks, W) + b[None,:,None,None]
            out = where(mask[:,None,:,:], 0.0, out)
shapes:     stacks [16,16,512,512] f32, mask [16,512,512] bool,
            W [8,16] f32, b [8] f32  ->  out [16,8,512,512] f32

Data-parallel over batch: 2 graphs per core x 8 cores.

The host shards AND re-lays-out the inputs into the exact on-chip tile
layouts so every DMA is fully contiguous (strided s-gather reads cap at
~200GB/s on TRN2 vs ~355GB/s contiguous; same HBM bytes either way).
The boolean mask is pre-broadcast over h on the host (uint8) so masking
is a plain elementwise multiply -- no broadcast matmuls on-chip.

Per graph, i in 4 superblocks w of 128 rows; i = 128w + 16*ih + il,
il = 8*c1 + i_in (c1 in {0,1}, i_in in [0,8)); cd = 2*ih + c1:

  rhs tile  [128,8192] f32 per (b,w): p = 8s+ih, f = il*512+j
      (one fully contiguous 4MB DMA, alternating the two HWDGE rings
       so two reads stay in flight and HBM latency spikes are hidden)
  keep tile [128,4096] u8 per (b,w): p = 8cd+h, f = i_in*512+j (512KB)
  psum [128,512] per (w,i_in): p = 8cd+h; two zero-padded-lhsT matmuls
      accumulate (c1=0,1): lhsT w_bd[8s+ih, 128c1 + 8(2ih+c1)+h] = W[h,s]
  epilogue: one DVE op: out = (psum + bias) * keep
  out tile [128,4096] f32 per (b,w): p = 8cd+h, f = i_in*512+j
      (one 2MB DMA on the SWDGE ring; 16KB h-strided runs write at
       line rate and writes tolerate the SWDGE issue latency)
Matmuls run as float32r so no input cast is needed (measured rel err
~1.4e-4 vs the f32 reference; the PE streams ~1 col/ns either dtype).
"""

import numpy as np
import ml_dtypes

B, S, N, H = 16, 16, 512, 8
NCORES = 8
BPC = B // NCORES  # graphs per core

MODE = "f32r"  # "f32r" | "bf16"

_CACHE = {}


def _build():
    import concourse.bacc as bacc
    import concourse.mybir as mybir
    import concourse.tile as tile

    f32 = mybir.dt.float32
    bf16 = mybir.dt.bfloat16
    cdt = mybir.dt.float32r if MODE == "f32r" else bf16

    nc = bacc.Bacc("TRN2", target_bir_lowering=False, debug=False,
                   num_devices=NCORES)

    # host-relaid stacks: [b, w, p=8s+ih, f=il*512+j]
    srl = nc.dram_tensor("srl", [BPC, 4, 128, 8192],
                         cdt if MODE == "f32r" else f32,
                         kind="ExternalInput")
    # host-broadcast keep mask: [b, w, p=8cd+h, f=i_in*512+j] uint8
    krl = nc.dram_tensor("krl", [BPC, 4, 128, 4096], mybir.dt.uint8,
                         kind="ExternalInput")
    w_bd = nc.dram_tensor("w_bd", [128, 256], cdt, kind="ExternalInput")
    bias = nc.dram_tensor("bias", [128, 1], f32, kind="ExternalInput")
    out = nc.dram_tensor("out", [BPC, H, N, N], f32, kind="ExternalOutput")

    # out per (b, w): [cd(16), h(8), (i_in j)(4096)]
    oview2 = out.ap().rearrange("b h (w cd iin) j -> b w cd h (iin j)",
                                w=4, cd=16, iin=8)

    ADD = mybir.AluOpType.add
    MULT = mybir.AluOpType.mult

    with tile.TileContext(nc) as tc:
        with (
            tc.tile_pool(name="const", bufs=1) as cpool,
            tc.tile_pool(name="maskp", bufs=2) as mpool,
            tc.tile_pool(name="data", bufs=4) as dpool,
            tc.tile_pool(name="outp", bufs=2) as opool,
            tc.tile_pool(name="psd", bufs=8, space="PSUM") as psd_pool,
        ):
            wbd_t = cpool.tile([128, 256], cdt)
            nc.sync.dma_start(wbd_t[:], w_bd.ap())
            bias_t = cpool.tile([128, 1], f32)
            nc.sync.dma_start(bias_t[:], bias.ap())

            for bb in range(BPC):
                for w in range(4):
                    rhs_t = dpool.tile([128, 8192], cdt, tag="rhs")
                    reng = nc.sync if (bb * 4 + w) % 2 == 0 else nc.scalar
                    if bb == 0 and w == 0:
                        # chunked first load in c1-paired order so the
                        # first psums unblock after ~2MB instead of 4MB
                        for fsl in (0, 4096, 2048, 6144):
                            reng.dma_start(
                                rhs_t[:, fsl:fsl + 2048],
                                srl.ap()[bb, w][:, fsl:fsl + 2048])
                    else:
                        reng.dma_start(rhs_t[:], srl.ap()[bb, w])
                    mask_t = mpool.tile([128, 4096], mybir.dt.uint8, tag="mask")
                    nc.sync.dma_start(mask_t[:], krl.ap()[bb, w])
                    out_t = opool.tile([128, 4096], f32)
                    for i_in in range(8):
                        ps_d = psd_pool.tile([128, 512], f32)
                        for c1 in range(2):
                            fsl = (8 * c1 + i_in) * 512
                            nc.tensor.matmul(
                                ps_d[:, :],
                                wbd_t[:, c1 * 128:c1 * 128 + 128],
                                rhs_t[:, fsl:fsl + 512],
                                start=(c1 == 0), stop=(c1 == 1))
                        # out = (ps_d + bias) * keep
                        nc.vector.scalar_tensor_tensor(
                            out_t[:, i_in * 512:i_in * 512 + 512], ps_d[:],
                            bias_t[:],
                            mask_t[:, i_in * 512:i_in * 512 + 512],
                            op0=ADD, op1=MULT)
                    nc.gpsimd.dma_start(oview2[bb, w], out_t[:])

    nc.compile()
    return nc


def _prep_consts(W, b):
    # lhsT for the c1-th accumulating matmul lives in w_bd[:, 128*c1:...]
    # w_bd[8s+ih, 128*c1 + 8*(2ih+c1) + h] = W[h, s]; rest zero.
    w_bd = np.zeros((128, 256), dtype=np.float32)
    for c1 in range(2):
        for ih in range(8):
            base = 128 * c1 + 8 * (2 * ih + c1)
            for h in range(8):
                w_bd[ih::8, base + h] = W[h, :]  # rows k = 8s+ih
    bias = np.tile(np.asarray(b, np.float32), 16).reshape(128, 1)
    if MODE == "bf16":
        w_bd = w_bd.astype(ml_dtypes.bfloat16)
    return w_bd, bias


def _relayout(stacks, mask):
    # srl[b, w, 8s+ih, il*512+j] = stacks[b, s, 128w+16ih+il, j]
    srl = stacks.reshape(B, S, 4, 8, 16, N)          # b s w ih il j
    srl = np.ascontiguousarray(srl.transpose(0, 2, 1, 3, 4, 5))
    srl = srl.reshape(B, 4, 128, 8192)
    # krl[b, w, 8cd+h, i_in*512+j] = 1 - mask[b, 128w+8cd+i_in, j]
    keep = (~np.asarray(mask, bool)).reshape(B, 4, 16, 8, N)  # b w cd iin j
    krl = np.broadcast_to(keep[:, :, :, None, :, :],
                          (B, 4, 16, 8, 8, N))                # b w cd h iin j
    krl = np.ascontiguousarray(krl.astype(np.uint8))
    krl = krl.reshape(B, 4, 128, 4096)
    return srl, krl


def kernel(stacks, mask, W, b):
    from concourse.bass_utils import run_bass_kernel_spmd

    if "nc" not in _CACHE:
        _CACHE["nc"] = _build()
    nc = _CACHE["nc"]

    stacks = np.asarray(stacks, dtype=np.float32)
    srl, krl = _relayout(stacks, np.asarray(mask))
    w_bd, bias = _prep_consts(np.asarray(W, np.float32),
                              np.asarray(b, np.float32))

    in_maps = []
    for c in range(NCORES):
        in_maps.append({
            "srl": srl[c * BPC:(c + 1) * BPC],
            "krl": krl[c * BPC:(c + 1) * BPC],
            "w_bd": w_bd, "bias": bias,
        })

    res = run_bass_kernel_spmd(nc, in_maps, core_ids=list(range(NCORES)),
                               **_CACHE.get("run_kwargs", {}))
    _CACHE["last_result"] = res
    outs = [r["out"] for r in res.results]
    return np.concatenate(outs, axis=0)
